# revision 32
# baseline (speedup 1.0000x reference)
"""Trainium2 Bass kernel: batched attention (B=8, S=4096, D=64), fp32.

out[b] = softmax(q[b] @ k[b].T / sqrt(D), axis=keys) @ v[b] * mask[b, :, None]

Sharding: data-parallel over the batch dim — one batch element per NeuronCore,
8 cores. Each core runs an identical single-core program on its own slice.

Per-core algorithm (matmul operands fp16; PSUM accumulation fp32):
  1. Piecewise prologue: input tensors are DMA'd in pieces; Q/K pieces are
     PE-transposed to d-major fp16 tiles (duplicated into partitions 64-127
     for row-tiled concurrent matmuls) as they arrive, so the first QK^T
     chunk starts ~6us in instead of waiting for the full load. A dummy
     exp at t=0 pulls the ACT table load off the critical path. Late pieces
     (K half 1, Q tiles 4-31) are transposed between early main-loop chunks.
  2. Main loop over 128 chunks (chunk = 2 k-tiles x 512 queries):
     scoresT into a 2-bank PSUM tile (two one-shot half-array matmuls, the
     even/odd k-tiles on the two 64-row halves of the PE array run
     concurrently), then ScalarE reads the PSUM pair directly:
     PT = exp(0.125 * scoresT) -> SBUF fp16. ACT does nothing but exp.
  3. PV trails two chunks behind: outT[65, q] += [V_kt | ones]^T @ PT_kt,
     full-row accumulating chain per q-chunk (the ones column makes row 64
     the softmax denominator). One-shot half-row interlopers inside the
     full-row accumulation chain are HW-verified safe.
  4. Epilogue per q-chunk: drain PV PSUM, PE-transpose back to [q, d],
     fuse *mask/denom on DVE, DMA out. Transpose staging shares the PSUM
     pool used by the prologue transposes (8 banks total: 2x2 scores +
     2x1 PV + 2x1 shared staging).
"""

import sys

if "/opt/trn_rl_repo" not in sys.path:
    sys.path.insert(0, "/opt/trn_rl_repo")

from contextlib import ExitStack

import numpy as np

import concourse.bass as bass
import concourse.mybir as mybir
import concourse.tile as tile
from concourse import bacc
from concourse.masks import make_identity

F32 = mybir.dt.float32
FP16 = mybir.dt.float16

B = 8          # batch == number of cores
S = 4096       # sequence length
D = 64         # head dim
P = 128        # partitions
NKT = S // P   # 32 k-tiles of 128 keys
QCHUNK = 512   # query chunk (one PSUM bank of fp32 per matmul)
NQC = S // QCHUNK          # 8 query chunks
NCH = 11                   # chunks per q-chunk: [3]*10 + [2] k-tiles
TOT = NQC * NCH            # 88 chunks total
SCALE = 1.0 / 8.0          # 1/sqrt(D)
PVLAG = 4                  # chunks PV trails behind QK^T+exp (2 pairs)

# Dual-Schraudolph crude exp (DVE+gpsimd) for a subset of chunks, offloading
# the saturated ACT engine.  exp(x) ~= B(round(x*A + B1)) + CW*B(round(x*A +
# B2)) where B() is int16-bitcast-to-fp16; the two phase-shifted sawtooth
# approximations average to +-1.0% multiplicative ripple, normalized so crude
# and exact slabs share the softmax denominator (numpy-validated: attention
# rel err 3.8e-3 vs 2e-2 gate).  A includes the 1/sqrt(D) score scale.
EXPA = 1024.0 / np.log(2.0) * SCALE
EXPB1 = 14277.1
EXPB2 = 13765.1
EXPCW = 1.4142135


CRUDE_EXP = False  # measured net-negative: ACT bubbles at crude chunks are
                   # not absorbable with only 2 sc PSUM slots of lookahead
                   # (HW: 163.9us vs 155.0us without; rel err 3.3e-3 vs 7.7e-4)


def is_crude(c):
    """chunks whose exp runs on DVE instead of ACT"""
    return CRUDE_EXP and c >= 12 and c % 4 == 3 and c % NCH != NCH - 1


def chunk_slabs(rel):
    """k-tile range of chunk `rel` within a q-chunk: [3]*10 + [2]."""
    return 3 * rel, (3 if rel < 10 else 2)


def build_attention(ctx: ExitStack, tc: tile.TileContext,
                    q_ap, k_ap, v_ap, mask_ap, out_ap):
    nc = tc.nc

    const_pool = ctx.enter_context(tc.tile_pool(name="const", bufs=1))
    io_pool = ctx.enter_context(tc.tile_pool(name="io", bufs=1))

    ident = const_pool.tile([P, P], F32, tag="ident", name="ident")
    make_identity(nc, ident)

    # ---- persistent SBUF tensors -------------------------------------------
    qt = [io_pool.tile([P, S // 2], FP16, tag=f"qt{h}", name=f"qt{h}")
          for h in range(2)]
    kt = [io_pool.tile([P, S // 2], FP16, tag=f"kt{h}", name=f"kt{h}")
          for h in range(2)]
    vp = io_pool.tile([P, NKT, D + 1], FP16, tag="vp", name="vp")

    qn = io_pool.tile([P, NKT, D], F32, tag="qn", name="qn")
    kn = io_pool.tile([P, NKT, D], F32, tag="kn", name="kn")
    vn = io_pool.tile([P, NKT, D], F32, tag="vn", name="vn")
    ones = io_pool.tile([P, NKT], F32, tag="ones", name="ones")

    q_tiled = q_ap.rearrange("(t p) d -> p t d", p=P)
    k_tiled = k_ap.rearrange("(t p) d -> p t d", p=P)
    v_tiled = v_ap.rearrange("(t p) d -> p t d", p=P)
    out_tiled = out_ap.rearrange("(t p) d -> p t d", p=P)

    # ---- pools -------------------------------------------------------------
    # PSUM: sc 2x3 banks + pv 1 + tps 1 (hook staging + epilogue, same tag
    # and slot size) = 8 banks exactly.
    sc_pool = ctx.enter_context(tc.tile_pool(name="sc", bufs=2, space="PSUM"))
    pv_pool = ctx.enter_context(tc.tile_pool(name="pv", bufs=1, space="PSUM"))
    tps_pool = ctx.enter_context(tc.tile_pool(name="tps", bufs=1, space="PSUM"))
    pt_pool = ctx.enter_context(tc.tile_pool(name="pt", bufs=7))
    sc16_pool = ctx.enter_context(tc.tile_pool(name="sc16", bufs=2))
    e_pool = ctx.enter_context(tc.tile_pool(name="ei", bufs=2))
    outt_pool = ctx.enter_context(tc.tile_pool(name="outt", bufs=2))
    osb_pool = ctx.enter_context(tc.tile_pool(name="osb", bufs=2))
    scal_pool = ctx.enter_context(tc.tile_pool(name="scal", bufs=4))

    # ---- prologue ----------------------------------------------------------
    nc.gpsimd.memset(ones, 1.0)
    # dummy exp: triggers the ACT table load at t=0; exp(0*1)=1.0 keeps the
    # ones tile correct and alive (no DCE).
    nc.scalar.activation(ones[0:1, 0:1], ones[0:1, 0:1],
                         mybir.ActivationFunctionType.Exp, scale=0.0)
    nc.vector.tensor_copy(vp[:, :, D], ones)

    # Input DMA queues: the SP queue carries ONLY K/Q pieces (plus the two
    # first-piece dups inline) in need order — anything else issued behind
    # them would head-block behind ~15us of input transfer. V rides the
    # gpsimd SWDGE queue in parallel; so do the late dups / mask / out.
    nc.sync.dma_start(kn[:, 0:4, :], k_tiled[:, 0:4, :])
    nc.sync.dma_start(qn[:, 0:4, :], q_tiled[:, 0:4, :])
    nc.sync.dma_start(kn[:, 4:8, :], k_tiled[:, 4:8, :])

    def dup(dst_halves, t0, t1, eng):
        """duplicate d-major cols of tiles [t0, t1) into partitions 64-127"""
        h0, c0 = divmod(t0 * P, S // 2)
        h1, c1 = divmod((t1 - 1) * P + P - 1, S // 2)
        assert h0 == h1, "dup must stay within one half"
        eng.dma_start(dst_halves[h0][D:P, c0:c1 + 1],
                      dst_halves[h0][0:D, c0:c1 + 1])

    # First pieces in strict need order (chunk 0 needs K tiles 0-2 and
    # Q 0-3): small transpose batches, drains alternating ACT (idle until
    # the first exp) / DVE, each followed by its own dup so the first QK^T
    # unblocks as early as possible. kn[8:16] is queued before the dups so
    # its transfer isn't head-blocked behind their drain waits.
    nc.sync.dma_start(kn[:, 8:16, :], k_tiled[:, 8:16, :])
    ps = sc_pool.tile([D, 6 * P], F32, tag="sc", name="ps")
    for j in range(3):
        nc.tensor.transpose(ps[:, j * P:(j + 1) * P], kn[:, j, :], ident)
    nc.scalar.copy(kt[0][0:D, 0:3 * P], ps[:, 0:3 * P])
    dup(kt, 0, 3, nc.sync)
    ps = sc_pool.tile([D, 6 * P], F32, tag="sc", name="ps")
    for j in range(4):
        nc.tensor.transpose(ps[:, j * P:(j + 1) * P], qn[:, j, :], ident)
    nc.vector.tensor_copy(qt[0][0:D, 0:4 * P], ps[:, 0:4 * P])
    dup(qt, 0, 4, nc.sync)
    ps = sc_pool.tile([D, 6 * P], F32, tag="sc", name="ps")
    for j in range(5):
        nc.tensor.transpose(ps[:, j * P:(j + 1) * P], kn[:, 3 + j, :], ident)
    nc.scalar.copy(kt[0][0:D, 3 * P:8 * P], ps[:, 0:5 * P])
    dup(kt, 3, 8, nc.sync)
    # V DMAs are emitted from hook c=1 (not here): emission position sets
    # scheduler priority, and issuing them first would let the gpsimd queue
    # run them ahead of the identity-matrix build that the first transposes
    # wait on. PV(0) doesn't execute until well after chunk 4, so V has
    # plenty of slack.

    def hook_transpose4(src, dst_halves, t0, next_dma=None):
        """one 4-tile transpose batch + drain + dup on SP, then (optionally)
        the next input piece DMA behind it on the SP queue"""
        ps = tps_pool.tile([D, 4 * P], F32, tag="tps", name="tps")
        for j in range(4):
            nc.tensor.transpose(ps[:, j * P:(j + 1) * P], src[:, t0 + j, :],
                                ident)
        half, dcol = divmod(t0 * P, S // 2)
        nc.vector.tensor_copy(dst_halves[half][0:D, dcol:dcol + 4 * P], ps)
        dup(dst_halves, t0, t0 + 4, nc.sync)
        if next_dma is not None:
            dst, src_d = next_dma
            nc.sync.dma_start(dst, src_d)

    def emit_vp_piece(t0, t1):
        nc.vector.tensor_copy(vp[:, t0:t1, 0:D], vn[:, t0:t1, :])

    def emit_v_dmas():
        nc.gpsimd.dma_start(vn[:, 0:4, :], v_tiled[:, 0:4, :])
        nc.gpsimd.dma_start(vn[:, 4:8, :], v_tiled[:, 4:8, :])
        nc.gpsimd.dma_start(vn[:, 8:16, :], v_tiled[:, 8:16, :])
        nc.gpsimd.dma_start(vn[:, 16:32, :], v_tiled[:, 16:32, :])
        nc.vector.tensor_copy(vp[:, 0:4, 0:D], vn[:, 0:4, :])

    # chunk index -> late input work to emit just before that chunk
    hooks = {
        1: lambda: (emit_v_dmas(), hook_transpose4(
            kn, kt, 8, (kn[:, 16:24, :], k_tiled[:, 16:24, :]))),
        2: lambda: (hook_transpose4(kn, kt, 12), emit_vp_piece(4, 8)),
        3: lambda: hook_transpose4(
            kn, kt, 16, (kn[:, 24:32, :], k_tiled[:, 24:32, :])),
        4: lambda: (hook_transpose4(kn, kt, 20), emit_vp_piece(8, 12)),
        5: lambda: (hook_transpose4(kn, kt, 24), emit_vp_piece(12, 16)),
        6: lambda: hook_transpose4(
            kn, kt, 28, (qn[:, 4:16, :], q_tiled[:, 4:16, :])),
        7: lambda: emit_vp_piece(16, 24),
        8: lambda: hook_transpose4(qn, qt, 4),
        9: lambda: (hook_transpose4(qn, qt, 8), emit_vp_piece(24, 32)),
        10: lambda: hook_transpose4(
            qn, qt, 12, (qn[:, 16:32, :], q_tiled[:, 16:32, :])),
        30: lambda: hook_transpose4(qn, qt, 16),
        32: lambda: hook_transpose4(qn, qt, 20),
        34: lambda: hook_transpose4(qn, qt, 24),
        36: lambda: hook_transpose4(qn, qt, 28),
    }

    # ---- main loop ---------------------------------------------------------
    pt_tiles = {}      # chunk index -> ptt tile
    pv_tiles = {}      # qc -> pv psum tile
    outt_tiles = {}    # qc -> outt staging tile (row 65 = mask)

    def emit_qkt_exp(c):
        qc, rel = divmod(c, NCH)
        k0, ln = chunk_slabs(rel)
        qt_half = qt[qc // 4]
        qcol = (qc % 4) * QCHUNK
        scs = sc_pool.tile([P, 3 * QCHUNK], F32, tag="sc", name="sc")
        for j in range(ln):
            k_tile = k0 + j
            h = k_tile % 2
            kt_half = kt[k_tile // (NKT // 2)]
            kcol = (k_tile * P) % (S // 2)
            nc.tensor.matmul(
                scs[:, j * QCHUNK:(j + 1) * QCHUNK],
                lhsT=kt_half[h * D:(h + 1) * D, kcol:kcol + P],
                rhs=qt_half[h * D:(h + 1) * D, qcol:qcol + QCHUNK],
                start=True, stop=True,
            )
        ptt = pt_pool.tile([P, 3 * QCHUNK], FP16, tag="ptt", name="ptt")
        pt_tiles[c] = ptt
        n = ln * QCHUNK
        if is_crude(c):
            # DVE: stage scores to fp16 SBUF (frees the PSUM slot as fast as
            # ACT would), then two half-rate int16 affine passes; gpsimd does
            # the weighted add into PT. ACT is untouched.
            sc16 = sc16_pool.tile([P, 3 * QCHUNK], FP16, tag="sc16",
                                  name="sc16")
            nc.vector.tensor_copy(sc16[:, 0:n], scs[:, 0:n])
            e1 = e_pool.tile([P, 3 * QCHUNK], mybir.dt.int16, tag="e1",
                             name="e1")
            e2 = e_pool.tile([P, 3 * QCHUNK], mybir.dt.int16, tag="e2",
                             name="e2")
            nc.vector.tensor_scalar(e1[:, 0:n], sc16[:, 0:n], EXPA, EXPB1,
                                    mybir.AluOpType.mult,
                                    mybir.AluOpType.add)
            nc.vector.tensor_scalar(e2[:, 0:n], sc16[:, 0:n], EXPA, EXPB2,
                                    mybir.AluOpType.mult,
                                    mybir.AluOpType.add)
            nc.vector.scalar_tensor_tensor(
                ptt[:, 0:n], e2.bitcast(FP16)[:, 0:n], EXPCW,
                e1.bitcast(FP16)[:, 0:n],
                mybir.AluOpType.mult, mybir.AluOpType.add)
        else:
            nc.scalar.activation(ptt[:, 0:n], scs[:, 0:n],
                                 mybir.ActivationFunctionType.Exp,
                                 scale=SCALE)

    def emit_prefetch(qc):
        outt = outt_pool.tile([D + 2, QCHUNK], F32, tag="outt", name="outt")
        outt_tiles[qc] = outt
        q0 = qc * QCHUNK
        nc.gpsimd.dma_start(outt[D + 1:D + 2, :], mask_ap[:, q0:q0 + QCHUNK])

    def emit_pv(c, first, last):
        qc, rel = divmod(c, NCH)
        k0, ln = chunk_slabs(rel)
        ptt = pt_tiles.pop(c)
        if first:
            pv_tiles[qc] = pv_pool.tile([D + 1, QCHUNK], F32, tag="pv",
                                        name="pv")
        pv_ps = pv_tiles[qc]
        for j in range(ln):
            nc.tensor.matmul(
                pv_ps[:],
                lhsT=vp[:, k0 + j, :],
                rhs=ptt[:, j * QCHUNK:(j + 1) * QCHUNK],
                start=(first and j == 0), stop=(last and j == ln - 1),
                skip_group_check=True,
            )

    def emit_epilogue_drain(qc):
        pv_ps = pv_tiles.pop(qc)
        outt = outt_tiles[qc]
        nc.vector.tensor_copy(outt[0:D + 1, :], pv_ps[:])

    def emit_epilogue(qc):
        # deferred 2 chunks after the drain so the PE-queue transposes never
        # head-block on the DVE drain
        outt = outt_tiles.pop(qc)
        osb = osb_pool.tile([P, QCHUNK // P, D], F32, tag="osb", name="osb")
        # all 4 transposes into one 1-bank PSUM tile (slot shared with the
        # hook staging tag); DVE consumes slices as they land
        tp = tps_pool.tile([P, QCHUNK // P, P], F32, tag="tps", name="tps")
        for jj in range(QCHUNK // P):
            nc.tensor.transpose(tp[:, jj, 0:D + 2],
                                outt[:, jj * P:(jj + 1) * P],
                                ident[0:D + 2, 0:D + 2])
        rs = scal_pool.tile([P, QCHUNK // P, 2], F32, tag="rs", name="rs")
        nc.vector.reciprocal(rs[:, :, 0:1], tp[:, :, D:D + 1])
        nc.vector.tensor_mul(rs[:, :, 1:2], rs[:, :, 0:1],
                             tp[:, :, D + 1:D + 2])
        for jj in range(QCHUNK // P):
            nc.vector.tensor_scalar(
                osb[:, jj, :], tp[:, jj, 0:D], rs[:, jj, 1:2], None,
                mybir.AluOpType.mult,
            )
        # last q-chunk's output goes via the (by then idle) HWDGE SP queue:
        # shorter desc-gen than SWDGE, off the critical teardown path
        eng = nc.sync if qc == NQC - 1 else nc.gpsimd
        eng.dma_start(
            out_tiled[:, qc * (QCHUNK // P):(qc + 1) * (QCHUNK // P), :], osb)

    # Chunk-PAIR software pipeline: QK bursts of 6 MMs and PV bursts of 6
    # halve the number of QK<->PV stationary-swap transitions on the PE
    # (each costs ~160ns of SBUF-access latency on the first matmul).
    for p in range(TOT // 2 + PVLAG // 2 + 2):
        for c in (2 * p, 2 * p + 1):
            if c < TOT:
                if c in hooks:
                    hooks[c]()
                if c % NCH == 0:
                    emit_prefetch(c // NCH)
        for c in (2 * p, 2 * p + 1):
            if c < TOT:
                emit_qkt_exp(c)
        for c in (2 * p - PVLAG, 2 * p - PVLAG + 1):
            if 0 <= c < TOT:
                qc, rel = divmod(c, NCH)
                exact = [r for r in range(NCH) if not is_crude(qc * NCH + r)]
                crude = [r for r in range(NCH) if is_crude(qc * NCH + r)]
                if is_crude(c):
                    continue  # crude PVs run at the end of the q-chunk
                emit_pv(c, first=(rel == exact[0]),
                        last=(rel == exact[-1] and not crude))
                if rel == NCH - 1:
                    # q-chunk complete modulo crude chunks: their PT has had
                    # the whole q-chunk to come through DVE+gpsimd
                    for i, r in enumerate(crude):
                        emit_pv(qc * NCH + r, first=False,
                                last=(i == len(crude) - 1))
                    emit_epilogue_drain(qc)
                    if qc == NQC - 1:
                        # last q-chunk: no reason to defer, nothing follows
                        emit_epilogue(qc)
        for c in (2 * p - PVLAG - 2, 2 * p - PVLAG - 1):
            if 0 <= c < TOT and c % NCH == NCH - 1 and c // NCH != NQC - 1:
                emit_epilogue(c // NCH)


def build_program():
    nc = bacc.Bacc("TRN2", target_bir_lowering=False, debug=False,
                   num_devices=B)
    q = nc.declare_dram_parameter("q", [S, D], F32, isOutput=False).ap()
    k = nc.declare_dram_parameter("k", [S, D], F32, isOutput=False).ap()
    v = nc.declare_dram_parameter("v", [S, D], F32, isOutput=False).ap()
    mask = nc.declare_dram_parameter("mask", [1, S], F32, isOutput=False).ap()
    out = nc.declare_dram_parameter("out", [S, D], F32, isOutput=True).ap()

    with tile.TileContext(nc) as tc, ExitStack() as ctx:
        build_attention(ctx, tc, q, k, v, mask, out)
    nc.compile()
    return nc


_NC_CACHE = None


def _get_nc():
    global _NC_CACHE
    if _NC_CACHE is None:
        _NC_CACHE = build_program()
    return _NC_CACHE


def make_in_maps(q, k, v, mask):
    return [
        {
            "q": np.ascontiguousarray(q[b], dtype=np.float32),
            "k": np.ascontiguousarray(k[b], dtype=np.float32),
            "v": np.ascontiguousarray(v[b], dtype=np.float32),
            "mask": np.ascontiguousarray(mask[b][None, :], dtype=np.float32),
        }
        for b in range(B)
    ]


def kernel(q, k, v, mask, _trace=False, _trace_kwargs=None):
    from concourse.bass_utils import run_bass_kernel_spmd

    nc = _get_nc()
    res = run_bass_kernel_spmd(
        nc, make_in_maps(q, k, v, mask), list(range(B)),
        trace=_trace, **(_trace_kwargs or {}),
    )
    out = np.stack([res.results[b]["out"] for b in range(B)])
    if _trace:
        return out, res
    return out


if __name__ == "__main__":
    rng = np.random.default_rng(0)
    q = rng.standard_normal((B, S, D), dtype=np.float32)
    k = rng.standard_normal((B, S, D), dtype=np.float32)
    v = rng.standard_normal((B, S, D), dtype=np.float32)
    mask = np.ones((B, S), dtype=np.float32)
    out = kernel(q, k, v, mask)
    print("out", out.shape, out.dtype, float(np.abs(out).max()))


# revision 35
# speedup vs baseline: 1.0134x; 1.0134x over previous
"""Trainium2 Bass kernel: batched attention (B=8, S=4096, D=64), fp32.

out[b] = softmax(q[b] @ k[b].T / sqrt(D), axis=keys) @ v[b] * mask[b, :, None]

Sharding: data-parallel over the batch dim — one batch element per NeuronCore,
8 cores. Each core runs an identical single-core program on its own slice.

Per-core algorithm (matmul operands fp16; PSUM accumulation fp32):
  1. Piecewise prologue: input tensors are DMA'd in pieces; Q/K pieces are
     PE-transposed to d-major fp16 tiles (duplicated into partitions 64-127
     for row-tiled concurrent matmuls) as they arrive, so the first QK^T
     chunk starts ~6us in instead of waiting for the full load. A dummy
     exp at t=0 pulls the ACT table load off the critical path. Late pieces
     (K half 1, Q tiles 4-31) are transposed between early main-loop chunks.
  2. Main loop over 128 chunks (chunk = 2 k-tiles x 512 queries):
     scoresT into a 2-bank PSUM tile (two one-shot half-array matmuls, the
     even/odd k-tiles on the two 64-row halves of the PE array run
     concurrently), then ScalarE reads the PSUM pair directly:
     PT = exp(0.125 * scoresT) -> SBUF fp16. ACT does nothing but exp.
  3. PV trails two chunks behind: outT[65, q] += [V_kt | ones]^T @ PT_kt,
     full-row accumulating chain per q-chunk (the ones column makes row 64
     the softmax denominator). One-shot half-row interlopers inside the
     full-row accumulation chain are HW-verified safe.
  4. Epilogue per q-chunk: drain PV PSUM, PE-transpose back to [q, d],
     fuse *mask/denom on DVE, DMA out. Transpose staging shares the PSUM
     pool used by the prologue transposes (8 banks total: 2x2 scores +
     2x1 PV + 2x1 shared staging).
"""

import sys

if "/opt/trn_rl_repo" not in sys.path:
    sys.path.insert(0, "/opt/trn_rl_repo")

from contextlib import ExitStack

import numpy as np

import concourse.bass as bass
import concourse.mybir as mybir
import concourse.tile as tile
from concourse import bacc
from concourse.masks import make_identity

F32 = mybir.dt.float32
FP16 = mybir.dt.float16

B = 8          # batch == number of cores
S = 4096       # sequence length
D = 64         # head dim
P = 128        # partitions
NKT = S // P   # 32 k-tiles of 128 keys
QCHUNK = 512   # query chunk (one PSUM bank of fp32 per matmul)
NQC = S // QCHUNK          # 8 query chunks
NCH = 11                   # chunks per q-chunk: [3]*10 + [2] k-tiles
TOT = NQC * NCH            # 88 chunks total
SCALE = 1.0 / 8.0          # 1/sqrt(D)
PVLAG = 4                  # chunks PV trails behind QK^T+exp (2 pairs)

# Dual-Schraudolph crude exp (DVE+gpsimd) for a subset of chunks, offloading
# the saturated ACT engine.  exp(x) ~= B(round(x*A + B1)) + CW*B(round(x*A +
# B2)) where B() is int16-bitcast-to-fp16; the two phase-shifted sawtooth
# approximations average to +-1.0% multiplicative ripple, normalized so crude
# and exact slabs share the softmax denominator (numpy-validated: attention
# rel err 3.8e-3 vs 2e-2 gate).  A includes the 1/sqrt(D) score scale.
EXPA = 1024.0 / np.log(2.0) * SCALE
EXPB1 = 14277.1
EXPB2 = 13765.1
EXPCW = 1.4142135


CRUDE_EXP = False  # measured net-negative: ACT bubbles at crude chunks are
                   # not absorbable with only 2 sc PSUM slots of lookahead
                   # (HW: 163.9us vs 155.0us without; rel err 3.3e-3 vs 7.7e-4)


def is_crude(c):
    """chunks whose exp runs on DVE instead of ACT"""
    return CRUDE_EXP and c >= 12 and c % 4 == 3 and c % NCH != NCH - 1


def chunk_slabs(rel):
    """k-tile range of chunk `rel` within a q-chunk: [3]*10 + [2]."""
    return 3 * rel, (3 if rel < 10 else 2)


def build_attention(ctx: ExitStack, tc: tile.TileContext,
                    q_ap, k_ap, v_ap, mask_ap, out_ap):
    nc = tc.nc

    const_pool = ctx.enter_context(tc.tile_pool(name="const", bufs=1))
    io_pool = ctx.enter_context(tc.tile_pool(name="io", bufs=1))

    ident = const_pool.tile([P, P], F32, tag="ident", name="ident")
    make_identity(nc, ident)

    # ---- persistent SBUF tensors -------------------------------------------
    qt = [io_pool.tile([P, S // 2], FP16, tag=f"qt{h}", name=f"qt{h}")
          for h in range(2)]
    kt = [io_pool.tile([P, S // 2], FP16, tag=f"kt{h}", name=f"kt{h}")
          for h in range(2)]
    vp = io_pool.tile([P, NKT, D + 1], FP16, tag="vp", name="vp")

    qn = io_pool.tile([P, NKT, D], F32, tag="qn", name="qn")
    kn = io_pool.tile([P, NKT, D], F32, tag="kn", name="kn")
    vn = io_pool.tile([P, NKT, D], F32, tag="vn", name="vn")
    ones = io_pool.tile([P, NKT], F32, tag="ones", name="ones")

    q_tiled = q_ap.rearrange("(t p) d -> p t d", p=P)
    k_tiled = k_ap.rearrange("(t p) d -> p t d", p=P)
    v_tiled = v_ap.rearrange("(t p) d -> p t d", p=P)
    out_tiled = out_ap.rearrange("(t p) d -> p t d", p=P)

    # ---- pools -------------------------------------------------------------
    # PSUM: sc 2x3 banks + pv 1 + tps 1 (hook staging + epilogue, same tag
    # and slot size) = 8 banks exactly.
    sc_pool = ctx.enter_context(tc.tile_pool(name="sc", bufs=2, space="PSUM"))
    pv_pool = ctx.enter_context(tc.tile_pool(name="pv", bufs=1, space="PSUM"))
    tps_pool = ctx.enter_context(tc.tile_pool(name="tps", bufs=1, space="PSUM"))
    pt_pool = ctx.enter_context(tc.tile_pool(name="pt", bufs=7))
    sc16_pool = ctx.enter_context(tc.tile_pool(name="sc16", bufs=2))
    e_pool = ctx.enter_context(tc.tile_pool(name="ei", bufs=2))
    outt_pool = ctx.enter_context(tc.tile_pool(name="outt", bufs=2))
    osb_pool = ctx.enter_context(tc.tile_pool(name="osb", bufs=2))
    scal_pool = ctx.enter_context(tc.tile_pool(name="scal", bufs=4))

    # ---- prologue ----------------------------------------------------------
    nc.gpsimd.memset(ones, 1.0)
    # dummy exp: triggers the ACT table load at t=0; exp(0*1)=1.0 keeps the
    # ones tile correct and alive (no DCE).
    nc.scalar.activation(ones[0:1, 0:1], ones[0:1, 0:1],
                         mybir.ActivationFunctionType.Exp, scale=0.0)
    nc.vector.tensor_copy(vp[:, :, D], ones)

    # Input DMA queues: the SP queue carries ONLY K/Q pieces (plus the two
    # first-piece dups inline) in need order — anything else issued behind
    # them would head-block behind ~15us of input transfer. V rides the
    # gpsimd SWDGE queue in parallel; so do the late dups / mask / out.
    nc.sync.dma_start(kn[:, 0:4, :], k_tiled[:, 0:4, :])
    nc.sync.dma_start(qn[:, 0:4, :], q_tiled[:, 0:4, :])
    nc.sync.dma_start(kn[:, 4:8, :], k_tiled[:, 4:8, :])

    def dup(dst_halves, t0, t1, eng):
        """duplicate d-major cols of tiles [t0, t1) into partitions 64-127"""
        h0, c0 = divmod(t0 * P, S // 2)
        h1, c1 = divmod((t1 - 1) * P + P - 1, S // 2)
        assert h0 == h1, "dup must stay within one half"
        eng.dma_start(dst_halves[h0][D:P, c0:c1 + 1],
                      dst_halves[h0][0:D, c0:c1 + 1])

    # First pieces in strict need order (chunk 0 needs K tiles 0-2 and
    # Q 0-3): small transpose batches, drains alternating ACT (idle until
    # the first exp) / DVE, each followed by its own dup so the first QK^T
    # unblocks as early as possible. kn[8:16] is queued before the dups so
    # its transfer isn't head-blocked behind their drain waits.
    nc.sync.dma_start(kn[:, 8:16, :], k_tiled[:, 8:16, :])
    ps = sc_pool.tile([D, 6 * P], F32, tag="sc", name="ps")
    for j in range(3):
        nc.tensor.transpose(ps[:, j * P:(j + 1) * P], kn[:, j, :], ident)
    nc.scalar.copy(kt[0][0:D, 0:3 * P], ps[:, 0:3 * P])
    dup(kt, 0, 3, nc.sync)
    ps = sc_pool.tile([D, 6 * P], F32, tag="sc", name="ps")
    for j in range(4):
        nc.tensor.transpose(ps[:, j * P:(j + 1) * P], qn[:, j, :], ident)
    nc.vector.tensor_copy(qt[0][0:D, 0:4 * P], ps[:, 0:4 * P])
    dup(qt, 0, 4, nc.sync)
    ps = sc_pool.tile([D, 6 * P], F32, tag="sc", name="ps")
    for j in range(5):
        nc.tensor.transpose(ps[:, j * P:(j + 1) * P], kn[:, 3 + j, :], ident)
    nc.scalar.copy(kt[0][0:D, 3 * P:8 * P], ps[:, 0:5 * P])
    dup(kt, 3, 8, nc.sync)
    # V DMAs are emitted from hook c=1 (not here): emission position sets
    # scheduler priority, and issuing them first would let the gpsimd queue
    # run them ahead of the identity-matrix build that the first transposes
    # wait on. PV(0) doesn't execute until well after chunk 4, so V has
    # plenty of slack.

    def hook_transpose4(src, dst_halves, t0, next_dma=None):
        """one 4-tile transpose batch + drain + dup on SP, then (optionally)
        the next input piece DMA behind it on the SP queue"""
        ps = tps_pool.tile([D, 4 * P], F32, tag="tps", name="tps")
        for j in range(4):
            nc.tensor.transpose(ps[:, j * P:(j + 1) * P], src[:, t0 + j, :],
                                ident)
        half, dcol = divmod(t0 * P, S // 2)
        nc.vector.tensor_copy(dst_halves[half][0:D, dcol:dcol + 4 * P], ps)
        dup(dst_halves, t0, t0 + 4, nc.sync)
        if next_dma is not None:
            dst, src_d = next_dma
            nc.sync.dma_start(dst, src_d)

    def emit_vp_piece(t0, t1):
        nc.vector.tensor_copy(vp[:, t0:t1, 0:D], vn[:, t0:t1, :])

    def emit_v_dmas():
        nc.gpsimd.dma_start(vn[:, 0:4, :], v_tiled[:, 0:4, :])
        nc.gpsimd.dma_start(vn[:, 4:8, :], v_tiled[:, 4:8, :])
        nc.gpsimd.dma_start(vn[:, 8:16, :], v_tiled[:, 8:16, :])
        nc.gpsimd.dma_start(vn[:, 16:32, :], v_tiled[:, 16:32, :])
        nc.vector.tensor_copy(vp[:, 0:4, 0:D], vn[:, 0:4, :])

    # chunk index -> late input work to emit just before that chunk
    hooks = {
        1: lambda: (emit_v_dmas(), hook_transpose4(
            kn, kt, 8, (kn[:, 16:24, :], k_tiled[:, 16:24, :]))),
        2: lambda: (hook_transpose4(kn, kt, 12), emit_vp_piece(4, 8)),
        3: lambda: hook_transpose4(
            kn, kt, 16, (kn[:, 24:32, :], k_tiled[:, 24:32, :])),
        4: lambda: (hook_transpose4(kn, kt, 20), emit_vp_piece(8, 12)),
        5: lambda: (hook_transpose4(kn, kt, 24), emit_vp_piece(12, 16)),
        6: lambda: hook_transpose4(
            kn, kt, 28, (qn[:, 4:16, :], q_tiled[:, 4:16, :])),
        7: lambda: emit_vp_piece(16, 24),
        8: lambda: hook_transpose4(qn, qt, 4),
        9: lambda: (hook_transpose4(qn, qt, 8), emit_vp_piece(24, 32)),
        10: lambda: hook_transpose4(
            qn, qt, 12, (qn[:, 16:32, :], q_tiled[:, 16:32, :])),
        30: lambda: hook_transpose4(qn, qt, 16),
        32: lambda: hook_transpose4(qn, qt, 20),
        34: lambda: hook_transpose4(qn, qt, 24),
        36: lambda: hook_transpose4(qn, qt, 28),
    }

    # ---- main loop ---------------------------------------------------------
    pt_tiles = {}      # chunk index -> ptt tile
    pv_tiles = {}      # qc -> pv psum tile
    outt_tiles = {}    # qc -> outt staging tile (row 65 = mask)

    def emit_qkt_exp(c):
        qc, rel = divmod(c, NCH)
        k0, ln = chunk_slabs(rel)
        qt_half = qt[qc // 4]
        qcol = (qc % 4) * QCHUNK
        scs = sc_pool.tile([P, 3 * QCHUNK], F32, tag="sc", name="sc")
        for j in range(ln):
            k_tile = k0 + j
            # chunks 0-1 run entirely on array half 0: no dependency on the
            # dup DMAs (partitions 64-127), so the first QK^T fires straight
            # off the drains. PE is idle in the ramp; serial MMs cost nothing.
            h = 0 if c < 2 else k_tile % 2
            kt_half = kt[k_tile // (NKT // 2)]
            kcol = (k_tile * P) % (S // 2)
            nc.tensor.matmul(
                scs[:, j * QCHUNK:(j + 1) * QCHUNK],
                lhsT=kt_half[h * D:(h + 1) * D, kcol:kcol + P],
                rhs=qt_half[h * D:(h + 1) * D, qcol:qcol + QCHUNK],
                start=True, stop=True,
            )
        ptt = pt_pool.tile([P, 3 * QCHUNK], FP16, tag="ptt", name="ptt")
        pt_tiles[c] = ptt
        n = ln * QCHUNK
        if is_crude(c):
            # DVE: stage scores to fp16 SBUF (frees the PSUM slot as fast as
            # ACT would), then two half-rate int16 affine passes; gpsimd does
            # the weighted add into PT. ACT is untouched.
            sc16 = sc16_pool.tile([P, 3 * QCHUNK], FP16, tag="sc16",
                                  name="sc16")
            nc.vector.tensor_copy(sc16[:, 0:n], scs[:, 0:n])
            e1 = e_pool.tile([P, 3 * QCHUNK], mybir.dt.int16, tag="e1",
                             name="e1")
            e2 = e_pool.tile([P, 3 * QCHUNK], mybir.dt.int16, tag="e2",
                             name="e2")
            nc.vector.tensor_scalar(e1[:, 0:n], sc16[:, 0:n], EXPA, EXPB1,
                                    mybir.AluOpType.mult,
                                    mybir.AluOpType.add)
            nc.vector.tensor_scalar(e2[:, 0:n], sc16[:, 0:n], EXPA, EXPB2,
                                    mybir.AluOpType.mult,
                                    mybir.AluOpType.add)
            nc.vector.scalar_tensor_tensor(
                ptt[:, 0:n], e2.bitcast(FP16)[:, 0:n], EXPCW,
                e1.bitcast(FP16)[:, 0:n],
                mybir.AluOpType.mult, mybir.AluOpType.add)
        else:
            nc.scalar.activation(ptt[:, 0:n], scs[:, 0:n],
                                 mybir.ActivationFunctionType.Exp,
                                 scale=SCALE)

    def emit_prefetch(qc):
        outt = outt_pool.tile([D + 2, QCHUNK], F32, tag="outt", name="outt")
        outt_tiles[qc] = outt
        q0 = qc * QCHUNK
        nc.gpsimd.dma_start(outt[D + 1:D + 2, :], mask_ap[:, q0:q0 + QCHUNK])

    def emit_pv(c, first, last):
        qc, rel = divmod(c, NCH)
        k0, ln = chunk_slabs(rel)
        ptt = pt_tiles.pop(c)
        if first:
            pv_tiles[qc] = pv_pool.tile([D + 1, QCHUNK], F32, tag="pv",
                                        name="pv")
        pv_ps = pv_tiles[qc]
        for j in range(ln):
            nc.tensor.matmul(
                pv_ps[:],
                lhsT=vp[:, k0 + j, :],
                rhs=ptt[:, j * QCHUNK:(j + 1) * QCHUNK],
                start=(first and j == 0), stop=(last and j == ln - 1),
                skip_group_check=True,
            )

    def emit_epilogue_drain(qc):
        pv_ps = pv_tiles.pop(qc)
        outt = outt_tiles[qc]
        if qc == NQC - 1:
            # tail: ACT is done with exps — split the drain across ACT/DVE
            half = QCHUNK // 2
            nc.scalar.copy(outt[0:D + 1, 0:half], pv_ps[:, 0:half])
            nc.vector.tensor_copy(outt[0:D + 1, half:], pv_ps[:, half:])
        else:
            nc.vector.tensor_copy(outt[0:D + 1, :], pv_ps[:])

    def emit_epilogue(qc):
        # deferred 2 chunks after the drain so the PE-queue transposes never
        # head-block on the DVE drain
        outt = outt_tiles.pop(qc)
        osb = osb_pool.tile([P, QCHUNK // P, D], F32, tag="osb", name="osb")
        # all 4 transposes into one 1-bank PSUM tile (slot shared with the
        # hook staging tag); DVE consumes slices as they land
        tp = tps_pool.tile([P, QCHUNK // P, P], F32, tag="tps", name="tps")
        for jj in range(QCHUNK // P):
            nc.tensor.transpose(tp[:, jj, 0:D + 2],
                                outt[:, jj * P:(jj + 1) * P],
                                ident[0:D + 2, 0:D + 2])
        rs = scal_pool.tile([P, QCHUNK // P, 2], F32, tag="rs", name="rs")
        nc.vector.reciprocal(rs[:, :, 0:1], tp[:, :, D:D + 1])
        nc.vector.tensor_mul(rs[:, :, 1:2], rs[:, :, 0:1],
                             tp[:, :, D + 1:D + 2])
        t0 = qc * (QCHUNK // P)
        if qc == NQC - 1:
            # tail: per-block output DMAs on the idle SP queue, each fired as
            # soon as its block's scale completes — overlaps DVE with DMA
            for jj in range(QCHUNK // P):
                nc.vector.tensor_scalar(
                    osb[:, jj, :], tp[:, jj, 0:D], rs[:, jj, 1:2], None,
                    mybir.AluOpType.mult,
                )
                nc.sync.dma_start(out_tiled[:, t0 + jj:t0 + jj + 1, :],
                                  osb[:, jj:jj + 1, :])
        else:
            for jj in range(QCHUNK // P):
                nc.vector.tensor_scalar(
                    osb[:, jj, :], tp[:, jj, 0:D], rs[:, jj, 1:2], None,
                    mybir.AluOpType.mult,
                )
            nc.gpsimd.dma_start(
                out_tiled[:, t0:t0 + QCHUNK // P, :], osb)

    # Chunk-PAIR software pipeline: QK bursts of 6 MMs and PV bursts of 6
    # halve the number of QK<->PV stationary-swap transitions on the PE
    # (each costs ~160ns of SBUF-access latency on the first matmul).
    for p in range(TOT // 2 + PVLAG // 2 + 2):
        for c in (2 * p, 2 * p + 1):
            if c < TOT:
                if c in hooks:
                    hooks[c]()
                if c % NCH == 0:
                    emit_prefetch(c // NCH)
        for c in (2 * p, 2 * p + 1):
            if c < TOT:
                emit_qkt_exp(c)
        for c in (2 * p - PVLAG, 2 * p - PVLAG + 1):
            if 0 <= c < TOT:
                qc, rel = divmod(c, NCH)
                exact = [r for r in range(NCH) if not is_crude(qc * NCH + r)]
                crude = [r for r in range(NCH) if is_crude(qc * NCH + r)]
                if is_crude(c):
                    continue  # crude PVs run at the end of the q-chunk
                emit_pv(c, first=(rel == exact[0]),
                        last=(rel == exact[-1] and not crude))
                if rel == NCH - 1:
                    # q-chunk complete modulo crude chunks: their PT has had
                    # the whole q-chunk to come through DVE+gpsimd
                    for i, r in enumerate(crude):
                        emit_pv(qc * NCH + r, first=False,
                                last=(i == len(crude) - 1))
                    emit_epilogue_drain(qc)
                    if qc == NQC - 1:
                        # last q-chunk: no reason to defer, nothing follows
                        emit_epilogue(qc)
        for c in (2 * p - PVLAG - 2, 2 * p - PVLAG - 1):
            if 0 <= c < TOT and c % NCH == NCH - 1 and c // NCH != NQC - 1:
                emit_epilogue(c // NCH)


def build_program():
    nc = bacc.Bacc("TRN2", target_bir_lowering=False, debug=False,
                   num_devices=B)
    q = nc.declare_dram_parameter("q", [S, D], F32, isOutput=False).ap()
    k = nc.declare_dram_parameter("k", [S, D], F32, isOutput=False).ap()
    v = nc.declare_dram_parameter("v", [S, D], F32, isOutput=False).ap()
    mask = nc.declare_dram_parameter("mask", [1, S], F32, isOutput=False).ap()
    out = nc.declare_dram_parameter("out", [S, D], F32, isOutput=True).ap()

    with tile.TileContext(nc) as tc, ExitStack() as ctx:
        build_attention(ctx, tc, q, k, v, mask, out)
    nc.compile()
    return nc


_NC_CACHE = None


def _get_nc():
    global _NC_CACHE
    if _NC_CACHE is None:
        _NC_CACHE = build_program()
    return _NC_CACHE


def make_in_maps(q, k, v, mask):
    return [
        {
            "q": np.ascontiguousarray(q[b], dtype=np.float32),
            "k": np.ascontiguousarray(k[b], dtype=np.float32),
            "v": np.ascontiguousarray(v[b], dtype=np.float32),
            "mask": np.ascontiguousarray(mask[b][None, :], dtype=np.float32),
        }
        for b in range(B)
    ]


def kernel(q, k, v, mask, _trace=False, _trace_kwargs=None):
    from concourse.bass_utils import run_bass_kernel_spmd

    nc = _get_nc()
    res = run_bass_kernel_spmd(
        nc, make_in_maps(q, k, v, mask), list(range(B)),
        trace=_trace, **(_trace_kwargs or {}),
    )
    out = np.stack([res.results[b]["out"] for b in range(B)])
    if _trace:
        return out, res
    return out


if __name__ == "__main__":
    rng = np.random.default_rng(0)
    q = rng.standard_normal((B, S, D), dtype=np.float32)
    k = rng.standard_normal((B, S, D), dtype=np.float32)
    v = rng.standard_normal((B, S, D), dtype=np.float32)
    mask = np.ones((B, S), dtype=np.float32)
    out = kernel(q, k, v, mask)
    print("out", out.shape, out.dtype, float(np.abs(out).max()))
